# revision 11
# baseline (speedup 1.0000x reference)
"""Trainium2 Bass kernel for MixtralBlockSparseTop2MLP grouped-GEMM MoE.

Problem: 4096 rows (sorted by expert), 8 experts, hidden=1024, ffn=3584.
  out[r] = silu(x[r] @ W1g[e(r)]) * (x[r] @ W1u[e(r)]) @ W2[e(r)]

Sharding: tensor-parallel over the ffn dimension. Each of the 8 cores gets
a 448-channel slice of every expert's gate/up/down weights and computes a
partial output for ALL 4096 rows; the host sums the 8 partials. All cores
run the identical program (segment structure baked from rows_for_experts at
call time), so one SPMD NEFF serves all 8 cores with per-core weight data.

v3 schedule:
 - gemm2 flipped: stationary = w2 [chan, h-slice], moving = a [chan, rows],
   so ragged chunks cost actual-rows moving columns (no ceil-128 rounding)
   and the 64-channel k3 tile needs no zero padding.
 - equal-split chunking per expert (no tiny tail chunks).
 - x and out in flat chunk-major layouts sized exactly per chunk, so every
   DMA is one contiguous run per partition (128 descriptors, not 1024).
 - w1 loads exclusively on the sync HWDGE ring; w2 loads and output stores
   on scalar/gpsimd, so weight prefetch is never queued behind stores.

Compute dtype: bf16 matmul inputs with fp32 PSUM accumulation.
"""

import os
import sys

sys.path.insert(0, "/opt/trn_rl_repo")

import numpy as np
import ml_dtypes

E, R, H, F = 8, 1024 * 4, 1024, 3584
FC = F // 8          # 448 ffn channels per core
NCH = 512            # max row-chunk (PSUM bank = 512 fp32 cols)
P = 128
KO = H // P          # 8 k-tiles for gemm1
K2 = 4               # ceil(448/128) k-tiles for gemm2 (last has 64 chans)
HT = H // P          # 8 output h-tiles for gemm2

BF16 = ml_dtypes.bfloat16

# test.py introspection: last BassKernelResults from run_bass_kernel_spmd
LAST_RESULT = None

_PROGRAM_CACHE = {}


def _segments(rows_for_experts):
    """[(expert, row_start, n_rows)] for experts with n_rows > 0."""
    segs = []
    r0 = 0
    for e in range(E):
        n = int(rows_for_experts[e])
        if n > 0:
            segs.append((e, r0, n))
        r0 += n
    # largest segment first (amortizes the prologue weight load),
    # smallest last (shortens the end-of-kernel pipeline drain).
    segs.sort(key=lambda s: -s[2])
    return segs


def _chunk_list(segments):
    """[(expert, row_start, nch, new_expert)] equal-split chunks in order."""
    out = []
    for (e, r0, n_e) in segments:
        k = (n_e + NCH - 1) // NCH
        base, rem = divmod(n_e, k)
        c0 = 0
        for i in range(k):
            nch = base + (1 if i < rem else 0)
            out.append((e, r0 + c0, nch, i == 0))
            c0 += nch
    return out


def _build_program(chunks, act_mode="silu"):
    import concourse.mybir as mybir
    import concourse.tile as tile
    from concourse import bacc

    dt = mybir.dt
    nc = bacc.Bacc(None, target_bir_lowering=False, debug=False)

    xT = nc.declare_dram_parameter("xT", [P, KO * R], dt.bfloat16, isOutput=False)
    w1 = nc.declare_dram_parameter("w1c", [E, P, KO, 2 * FC], dt.bfloat16, isOutput=False)
    w2 = nc.declare_dram_parameter("w2c", [E, P, K2, H], dt.bfloat16, isOutput=False)
    outp = nc.declare_dram_parameter("outp", [P, HT * R], dt.bfloat16, isOutput=True)

    silu = mybir.ActivationFunctionType.Silu
    sigmoid = mybir.ActivationFunctionType.Sigmoid

    with tile.TileContext(nc) as tc:
        with (
            tc.tile_pool(name="w1p", bufs=3) as w1p,
            tc.tile_pool(name="w2p", bufs=3) as w2p,
            tc.tile_pool(name="xp", bufs=4) as xp,
            tc.tile_pool(name="apool", bufs=2) as apool,
            tc.tile_pool(name="a3pool", bufs=1) as a3pool,
            tc.tile_pool(name="opool", bufs=3) as opool,
            tc.tile_pool(name="hps", bufs=4, space="PSUM") as hps,
            tc.tile_pool(name="ops", bufs=4, space="PSUM") as ops,
        ):
            # a3 holds unit 6 in rows 0:64; rows 64:128 multiply the exact-
            # zero padded w2 k3 rows, so they only need zeroing once (the
            # full 128-partition stationary keeps the PE's weight-load
            # pipelined; 64-partition stationaries cost ~105ns extra each).
            a3_tiles = [
                a3pool.tile([P, NCH], dt.bfloat16, tag=f"a3_{i}", name=f"a3_{i}")
                for i in range(2)
            ]
            for t3 in a3_tiles:
                nc.vector.memset(t3[:], 0.0)

            pending_gemm2 = None
            n_chunks = len(chunks)
            off = 0
            for ci, (e, r0, nch, new_expert) in enumerate(chunks):
                first = ci == 0
                xbase = KO * off
                if new_expert:
                    w1sb = w1p.tile([P, KO, 2 * FC], dt.bfloat16, tag="w1sb")
                    w2sb = w2p.tile([P, K2, H], dt.bfloat16, tag="w2sb")
                if first:
                    # prologue: split w1/x over both HWDGE rings in a few
                    # DMAs (<=8 in flight; 8 DMA tracking lanes), ordered to
                    # match the PE's k-major consumption of u-slice 0.
                    xsb = xp.tile([P, KO * nch], dt.bfloat16, tag="xsb")
                    nc.sync.dma_start(w1sb[:, 0, :], w1[e, :, 0, :])
                    nc.scalar.dma_start(
                        xsb[:, : 2 * nch], xT[:, xbase : xbase + 2 * nch]
                    )
                    nc.sync.dma_start(w1sb[:, 1:4, :], w1[e, :, 1:4, :])
                    nc.scalar.dma_start(
                        xsb[:, 2 * nch : 5 * nch],
                        xT[:, xbase + 2 * nch : xbase + 5 * nch],
                    )
                    nc.sync.dma_start(w1sb[:, 4:KO, :], w1[e, :, 4:KO, :])
                    nc.scalar.dma_start(
                        xsb[:, 5 * nch :],
                        xT[:, xbase + 5 * nch : xbase + KO * nch],
                    )
                    nc.scalar.dma_start(w2sb[:], w2[e])
                else:
                    if new_expert:
                        nc.sync.dma_start(w1sb[:, 0:4, :], w1[e, :, 0:4, :])
                        nc.sync.dma_start(w1sb[:, 4:KO, :], w1[e, :, 4:KO, :])
                        nc.scalar.dma_start(w2sb[:], w2[e])
                    xsb = xp.tile([P, KO * nch], dt.bfloat16, tag="xsb")
                    xeng = (nc.sync, nc.scalar)[ci % 2]
                    xeng.dma_start(xsb[:], xT[:, xbase : xbase + KO * nch])

                # gemm1: 7 packed m-slices [gate_u(64) | up_u(64)];
                # psum_u partitions 0:64 = gate, 64:128 = up.
                # silu via ACT into a 64-row tmp, then DVE cross-base
                # multiply into the packed a k-tiles.
                a_tiles = [
                    apool.tile([P, NCH], dt.bfloat16, tag=f"a{j}", name=f"a{j}")
                    for j in range(3)
                ]
                a3 = a3_tiles[ci % 2]
                for u in range(7):
                    hu_ps = hps.tile([P, NCH], dt.float32, tag="h", name=f"h{u}")
                    for k in range(KO):
                        nc.tensor.matmul(
                            hu_ps[:, :nch],
                            w1sb[:, k, P * u : P * u + P],
                            xsb[:, k * nch : k * nch + nch],
                            start=(k == 0),
                            stop=(k == KO - 1),
                        )
                    stmp = apool.tile([64, NCH], dt.bfloat16, tag="stmp", name="stmp")
                    if act_mode == "silu":
                        nc.scalar.activation(stmp[:, :nch], hu_ps[0:64, :nch], silu)
                    else:  # silu(g) = g * sigmoid(g); CoreSim lacks Silu
                        nc.scalar.activation(stmp[:, :nch], hu_ps[0:64, :nch], sigmoid)
                        nc.vector.tensor_mul(
                            stmp[:, :nch], stmp[:, :nch], hu_ps[0:64, :nch]
                        )
                    if u < 6:
                        lo = 64 * (u % 2)
                        dst = a_tiles[u // 2][lo : lo + 64, :nch]
                    else:
                        dst = a3[0:64, :nch]
                    nc.vector.tensor_mul(dst, stmp[:, :nch], hu_ps[64:128, :nch])

                # gemm2 (emitted one chunk behind gemm1 so the PE never waits
                # on this chunk's silu/mul chain), flipped orientation:
                # stationary = w2 [chan, 128 h], moving = a [chan, rows].
                # Output psum [128 h, rows]; cost scales with actual rows.
                def gemm2(
                    ci=ci, nch=nch, off=off, a_tiles=a_tiles, a3=a3, w2sb=w2sb
                ):
                    last = ci >= n_chunks - 2
                    osb = opool.tile([P, HT * nch], dt.bfloat16, tag="osb", name="osb")
                    for h in range(HT):
                        ot = ops.tile([P, NCH], dt.float32, tag="o", name=f"o{h}")
                        for k in range(K2):
                            lhsT = w2sb[:, k, P * h : P * h + P]
                            rhs = a_tiles[k][:, :nch] if k < 3 else a3[:, :nch]
                            nc.tensor.matmul(
                                ot[:, :nch],
                                lhsT,
                                rhs,
                                start=(k == 0),
                                stop=(k == K2 - 1),
                            )
                        # tail chunks: alternate DVE/ACT copies so the
                        # final psum drain isn't one serial DVE chain.
                        if last and h % 2:
                            nc.scalar.activation(
                                osb[:, h * nch : h * nch + nch],
                                ot[:, :nch],
                                mybir.ActivationFunctionType.Copy,
                            )
                        else:
                            nc.vector.tensor_copy(
                                osb[:, h * nch : h * nch + nch], ot[:, :nch]
                            )
                    if ci == n_chunks - 1:
                        eng = nc.scalar
                    elif ci == n_chunks - 2:
                        eng = nc.sync
                    else:
                        eng = nc.gpsimd
                    obase = HT * off
                    eng.dma_start(outp[:, obase : obase + HT * nch], osb[:])

                if pending_gemm2 is not None:
                    pending_gemm2()
                pending_gemm2 = gemm2
                off += nch
            pending_gemm2()

    nc.compile()
    return nc


def _prepare_inputs(hidden_states, w1, w2, chunks):
    """Host-side shard/layout/cast. Returns (xT, [w1c per core], [w2c per core])."""
    x = np.asarray(hidden_states, dtype=np.float32)
    w1 = np.asarray(w1, dtype=np.float32)
    w2 = np.asarray(w2, dtype=np.float32)

    xb = x.astype(BF16)          # [R, H]
    w1b = w1.astype(BF16)        # [E, H, 2F]
    w2b = w2.astype(BF16)        # [E, F, H]

    # flat chunk-major packed x: for chunk at row offset `off`,
    # xT[p, KO*off + k*nch + j] = x[r0 + j, 128k + p]
    xT = np.empty((P, KO * R), dtype=BF16)
    off = 0
    for (_, r0, nch, _) in chunks:
        blk = xb[r0 : r0 + nch].T.reshape(KO, P, nch).transpose(1, 0, 2)
        xT[:, KO * off : KO * (off + nch)] = blk.reshape(P, KO * nch)
        off += nch

    w1cs, w2cs = [], []
    for c in range(8):
        gate = w1b[:, :, c * FC : (c + 1) * FC]
        up = w1b[:, :, F + c * FC : F + (c + 1) * FC]
        # interleave 64-channel blocks: [G0|U0|G1|U1|...|G6|U6] so each
        # 128-column m-slice u packs gate_u in psum partitions 0:64 and
        # up_u in 64:128.
        w1cat = np.ascontiguousarray(
            np.stack(
                [gate.reshape(E, H, FC // 64, 64), up.reshape(E, H, FC // 64, 64)],
                axis=3,
            ).reshape(E, H, 2 * FC)
        )
        w1c = np.ascontiguousarray(
            w1cat.reshape(E, H // P, P, 2 * FC).transpose(0, 2, 1, 3)
        )
        # flipped gemm2 stationary layout: w2c[e, p, k, h] = w2[e, 128k+p, h]
        # (within this core's 448-channel slice; k3 rows 64:128 unused)
        w2pad = np.zeros((E, K2 * P, H), dtype=BF16)
        w2pad[:, :FC, :] = w2b[:, c * FC : (c + 1) * FC, :]
        w2c = np.ascontiguousarray(
            w2pad.reshape(E, K2, P, H).transpose(0, 2, 1, 3)
        )
        w1cs.append(w1c)
        w2cs.append(w2c)
    return xT, w1cs, w2cs


def kernel(hidden_states, w1, w2, rows_for_experts):
    global LAST_RESULT
    from concourse.bass_utils import run_bass_kernel_spmd

    segs = _segments(np.asarray(rows_for_experts))
    if not segs:
        return np.zeros((R, H), dtype=np.float32)
    chunks = _chunk_list(segs)
    key = tuple(chunks)
    nc = _PROGRAM_CACHE.get(key)
    if nc is None:
        nc = _build_program(chunks)
        _PROGRAM_CACHE[key] = nc

    xT, w1cs, w2cs = _prepare_inputs(hidden_states, w1, w2, chunks)
    in_maps = [
        {"xT": xT, "w1c": w1cs[c], "w2c": w2cs[c]} for c in range(8)
    ]
    res = run_bass_kernel_spmd(nc, in_maps, core_ids=list(range(8)))
    LAST_RESULT = res

    acc = np.zeros((R, H), dtype=np.float32)
    for c in range(8):
        blocks = res.results[c]["outp"]  # [P, HT*R] bf16, chunk-major cols
        off = 0
        for (_, r0, nch, _) in chunks:
            blk = blocks[:, HT * off : HT * (off + nch)].reshape(P, HT, nch)
            acc[r0 : r0 + nch] += blk.transpose(2, 1, 0).reshape(nch, H)
            off += nch
    return acc


# revision 15
# speedup vs baseline: 1.0629x; 1.0629x over previous
"""Trainium2 Bass kernel for MixtralBlockSparseTop2MLP grouped-GEMM MoE.

Problem: 4096 rows (sorted by expert), 8 experts, hidden=1024, ffn=3584.
  out[r] = silu(x[r] @ W1g[e(r)]) * (x[r] @ W1u[e(r)]) @ W2[e(r)]

Sharding: tensor-parallel over the ffn dimension. Each of the 8 cores gets
a 448-channel slice of every expert's gate/up/down weights and computes a
partial output for ALL 4096 rows; the host sums the 8 partials. All cores
run the identical program (segment structure baked from rows_for_experts at
call time), so one SPMD NEFF serves all 8 cores with per-core weight data.

v3 schedule:
 - gemm2 flipped: stationary = w2 [chan, h-slice], moving = a [chan, rows],
   so ragged chunks cost actual-rows moving columns (no ceil-128 rounding)
   and the 64-channel k3 tile needs no zero padding.
 - equal-split chunking per expert (no tiny tail chunks).
 - x and out in flat chunk-major layouts sized exactly per chunk, so every
   DMA is one contiguous run per partition (128 descriptors, not 1024).
 - w1 loads exclusively on the sync HWDGE ring; w2 loads and output stores
   on scalar/gpsimd, so weight prefetch is never queued behind stores.

Compute dtype: bf16 matmul inputs with fp32 PSUM accumulation.
"""

import os
import sys

sys.path.insert(0, "/opt/trn_rl_repo")

import numpy as np
import ml_dtypes

E, R, H, F = 8, 1024 * 4, 1024, 3584
FC = F // 8          # 448 ffn channels per core
NCH = 512            # max row-chunk (PSUM bank = 512 fp32 cols)
P = 128
KO = H // P          # 8 k-tiles for gemm1
K2 = 4               # ceil(448/128) k-tiles for gemm2 (last has 64 chans)
HT = H // P          # 8 output h-tiles for gemm2

BF16 = ml_dtypes.bfloat16

# test.py introspection: last BassKernelResults from run_bass_kernel_spmd
LAST_RESULT = None

_PROGRAM_CACHE = {}


def _segments(rows_for_experts):
    """[(expert, row_start, n_rows)] for experts with n_rows > 0."""
    segs = []
    r0 = 0
    for e in range(E):
        n = int(rows_for_experts[e])
        if n > 0:
            segs.append((e, r0, n))
        r0 += n
    # largest segment first (amortizes the prologue weight load),
    # smallest last (shortens the end-of-kernel pipeline drain).
    segs.sort(key=lambda s: -s[2])
    return segs


def _chunk_list(segments):
    """[(expert, row_start, nch, new_expert)] equal-split chunks in order."""
    out = []
    for (e, r0, n_e) in segments:
        k = (n_e + NCH - 1) // NCH
        base, rem = divmod(n_e, k)
        c0 = 0
        for i in range(k):
            nch = base + (1 if i < rem else 0)
            out.append((e, r0 + c0, nch, i == 0))
            c0 += nch
    return out


def _build_program(chunks, act_mode="silu"):
    import concourse.mybir as mybir
    import concourse.tile as tile
    from concourse import bacc

    dt = mybir.dt
    nc = bacc.Bacc(None, target_bir_lowering=False, debug=False)

    xT = nc.declare_dram_parameter("xT", [P, KO * R], dt.bfloat16, isOutput=False)
    w1 = nc.declare_dram_parameter("w1c", [E, P, KO, 2 * FC], dt.bfloat16, isOutput=False)
    w2 = nc.declare_dram_parameter("w2c", [E, P, K2, H], dt.bfloat16, isOutput=False)
    n_chunks = len(chunks)
    outp = nc.declare_dram_parameter(
        "outp", [n_chunks, P, NCH // P, H], dt.bfloat16, isOutput=True
    )

    silu = mybir.ActivationFunctionType.Silu
    sigmoid = mybir.ActivationFunctionType.Sigmoid

    with tile.TileContext(nc) as tc:
        with (
            tc.tile_pool(name="w1p", bufs=3) as w1p,
            tc.tile_pool(name="w2p", bufs=3) as w2p,
            tc.tile_pool(name="xp", bufs=4) as xp,
            tc.tile_pool(name="apool", bufs=2) as apool,
            tc.tile_pool(name="a3pool", bufs=1) as a3pool,
            tc.tile_pool(name="opool", bufs=3) as opool,
            tc.tile_pool(name="hps", bufs=4, space="PSUM") as hps,
            tc.tile_pool(name="ops", bufs=4, space="PSUM") as ops,
        ):
            # a3 holds unit 6 in rows 0:64; rows 64:128 multiply the exact-
            # zero padded w2 k3 rows, so they only need zeroing once (stale
            # SBUF garbage could be NaN, and NaN*0 = NaN).
            a3_tiles = [
                a3pool.tile([P, NCH], dt.bfloat16, tag=f"a3_{i}", name=f"a3_{i}")
                for i in range(2)
            ]
            for t3 in a3_tiles:
                nc.vector.memset(t3[:], 0.0)

            pending_gemm2 = None
            off = 0
            for ci, (e, r0, nch, new_expert) in enumerate(chunks):
                first = ci == 0
                xbase = KO * off
                if new_expert:
                    w1sb = w1p.tile([P, KO, 2 * FC], dt.bfloat16, tag="w1sb")
                    w2sb = w2p.tile([P, K2, H], dt.bfloat16, tag="w2sb")
                if first:
                    # prologue: split w1/x over both HWDGE rings in a few
                    # DMAs (<=8 in flight; 8 DMA tracking lanes), ordered to
                    # match the PE's k-major consumption of u-slice 0.
                    xsb = xp.tile([P, KO * nch], dt.bfloat16, tag="xsb")
                    nc.sync.dma_start(w1sb[:, 0, :], w1[e, :, 0, :])
                    nc.scalar.dma_start(
                        xsb[:, : 2 * nch], xT[:, xbase : xbase + 2 * nch]
                    )
                    nc.sync.dma_start(w1sb[:, 1:4, :], w1[e, :, 1:4, :])
                    nc.scalar.dma_start(
                        xsb[:, 2 * nch : 5 * nch],
                        xT[:, xbase + 2 * nch : xbase + 5 * nch],
                    )
                    nc.sync.dma_start(w1sb[:, 4:KO, :], w1[e, :, 4:KO, :])
                    nc.scalar.dma_start(
                        xsb[:, 5 * nch :],
                        xT[:, xbase + 5 * nch : xbase + KO * nch],
                    )
                    nc.scalar.dma_start(w2sb[:], w2[e])
                else:
                    if new_expert:
                        nc.sync.dma_start(w1sb[:, 0:4, :], w1[e, :, 0:4, :])
                        nc.sync.dma_start(w1sb[:, 4:KO, :], w1[e, :, 4:KO, :])
                        nc.scalar.dma_start(w2sb[:], w2[e])
                    xsb = xp.tile([P, KO * nch], dt.bfloat16, tag="xsb")
                    xeng = (nc.sync, nc.scalar)[ci % 2]
                    xeng.dma_start(xsb[:], xT[:, xbase : xbase + KO * nch])

                # gemm1: 7 packed m-slices [gate_u(64) | up_u(64)];
                # psum_u partitions 0:64 = gate, 64:128 = up.
                # silu via ACT into a 64-row tmp, then DVE cross-base
                # multiply into the packed a k-tiles.
                a_tiles = [
                    apool.tile([P, NCH], dt.bfloat16, tag=f"a{j}", name=f"a{j}")
                    for j in range(3)
                ]
                a3 = a3_tiles[ci % 2]
                for u in range(7):
                    hu_ps = hps.tile([P, NCH], dt.float32, tag="h", name=f"h{u}")
                    for k in range(KO):
                        nc.tensor.matmul(
                            hu_ps[:, :nch],
                            w1sb[:, k, P * u : P * u + P],
                            xsb[:, k * nch : k * nch + nch],
                            start=(k == 0),
                            stop=(k == KO - 1),
                        )
                    stmp = apool.tile([64, NCH], dt.bfloat16, tag="stmp", name="stmp")
                    if act_mode == "silu":
                        nc.scalar.activation(stmp[:, :nch], hu_ps[0:64, :nch], silu)
                    else:  # silu(g) = g * sigmoid(g); CoreSim lacks Silu
                        nc.scalar.activation(stmp[:, :nch], hu_ps[0:64, :nch], sigmoid)
                        nc.vector.tensor_mul(
                            stmp[:, :nch], stmp[:, :nch], hu_ps[0:64, :nch]
                        )
                    if u < 6:
                        lo = 64 * (u % 2)
                        dst = a_tiles[u // 2][lo : lo + 64, :nch]
                    else:
                        dst = a3[0:64, :nch]
                    nc.vector.tensor_mul(dst, stmp[:, :nch], hu_ps[64:128, :nch])

                # gemm2 (emitted one chunk behind gemm1 so the PE never waits
                # on this chunk's silu/mul chain): stationary = a [chan,
                # rows-slice], moving = w2 [chan, 512 H-cols]. 512-col
                # streams fully hide the per-matmul LDWEIGHTS (~100ns).
                def gemm2(
                    ci=ci, nch=nch, a_tiles=a_tiles, a3=a3, w2sb=w2sb
                ):
                    last = ci >= n_chunks - 2
                    osb = opool.tile(
                        [P, NCH // P, H], dt.bfloat16, tag="osb", name="osb"
                    )
                    for s0 in range(0, nch, P):
                        rows = min(P, nch - s0)
                        s = s0 // P
                        otiles = [
                            ops.tile([P, 512], dt.float32, tag="o", name=f"o{h2}")
                            for h2 in range(H // 512)
                        ]
                        for k in range(K2):
                            ak = a_tiles[k] if k < 3 else a3
                            for h2 in range(H // 512):
                                nc.tensor.matmul(
                                    otiles[h2][:rows, :],
                                    ak[:, s0 : s0 + rows],
                                    w2sb[:, k, 512 * h2 : 512 * h2 + 512],
                                    start=(k == 0),
                                    stop=(k == K2 - 1),
                                )
                        for h2 in range(H // 512):
                            # tail chunks: alternate DVE/ACT copies so the
                            # final psum drain isn't one serial DVE chain.
                            if last and h2 % 2:
                                nc.scalar.activation(
                                    osb[:rows, s, 512 * h2 : 512 * h2 + 512],
                                    otiles[h2][:rows, :],
                                    mybir.ActivationFunctionType.Copy,
                                )
                            else:
                                nc.vector.tensor_copy(
                                    osb[:rows, s, 512 * h2 : 512 * h2 + 512],
                                    otiles[h2][:rows, :],
                                )
                    if ci == n_chunks - 1:
                        eng = nc.scalar
                    elif ci == n_chunks - 2:
                        eng = nc.sync
                    else:
                        eng = nc.gpsimd
                    sf = nch // P
                    rem = nch - P * sf
                    if sf:
                        eng.dma_start(outp[ci][:, :sf, :], osb[:, :sf, :])
                    if rem:
                        eng.dma_start(outp[ci][:rem, sf, :], osb[:rem, sf, :])

                if pending_gemm2 is not None:
                    pending_gemm2()
                pending_gemm2 = gemm2
                off += nch
            pending_gemm2()

    nc.compile()
    return nc


def _prepare_inputs(hidden_states, w1, w2, chunks):
    """Host-side shard/layout/cast. Returns (xT, [w1c per core], [w2c per core])."""
    x = np.asarray(hidden_states, dtype=np.float32)
    w1 = np.asarray(w1, dtype=np.float32)
    w2 = np.asarray(w2, dtype=np.float32)

    xb = x.astype(BF16)          # [R, H]
    w1b = w1.astype(BF16)        # [E, H, 2F]
    w2b = w2.astype(BF16)        # [E, F, H]

    # flat chunk-major packed x: for chunk at row offset `off`,
    # xT[p, KO*off + k*nch + j] = x[r0 + j, 128k + p]
    xT = np.empty((P, KO * R), dtype=BF16)
    off = 0
    for (_, r0, nch, _) in chunks:
        blk = xb[r0 : r0 + nch].T.reshape(KO, P, nch).transpose(1, 0, 2)
        xT[:, KO * off : KO * (off + nch)] = blk.reshape(P, KO * nch)
        off += nch

    w1cs, w2cs = [], []
    for c in range(8):
        gate = w1b[:, :, c * FC : (c + 1) * FC]
        up = w1b[:, :, F + c * FC : F + (c + 1) * FC]
        # interleave 64-channel blocks: [G0|U0|G1|U1|...|G6|U6] so each
        # 128-column m-slice u packs gate_u in psum partitions 0:64 and
        # up_u in 64:128.
        w1cat = np.ascontiguousarray(
            np.stack(
                [gate.reshape(E, H, FC // 64, 64), up.reshape(E, H, FC // 64, 64)],
                axis=3,
            ).reshape(E, H, 2 * FC)
        )
        w1c = np.ascontiguousarray(
            w1cat.reshape(E, H // P, P, 2 * FC).transpose(0, 2, 1, 3)
        )
        # flipped gemm2 stationary layout: w2c[e, p, k, h] = w2[e, 128k+p, h]
        # (within this core's 448-channel slice; k3 rows 64:128 unused)
        w2pad = np.zeros((E, K2 * P, H), dtype=BF16)
        w2pad[:, :FC, :] = w2b[:, c * FC : (c + 1) * FC, :]
        w2c = np.ascontiguousarray(
            w2pad.reshape(E, K2, P, H).transpose(0, 2, 1, 3)
        )
        w1cs.append(w1c)
        w2cs.append(w2c)
    return xT, w1cs, w2cs


def kernel(hidden_states, w1, w2, rows_for_experts):
    global LAST_RESULT
    from concourse.bass_utils import run_bass_kernel_spmd

    segs = _segments(np.asarray(rows_for_experts))
    if not segs:
        return np.zeros((R, H), dtype=np.float32)
    chunks = _chunk_list(segs)
    key = tuple(chunks)
    nc = _PROGRAM_CACHE.get(key)
    if nc is None:
        nc = _build_program(chunks)
        _PROGRAM_CACHE[key] = nc

    xT, w1cs, w2cs = _prepare_inputs(hidden_states, w1, w2, chunks)
    in_maps = [
        {"xT": xT, "w1c": w1cs[c], "w2c": w2cs[c]} for c in range(8)
    ]
    res = run_bass_kernel_spmd(nc, in_maps, core_ids=list(range(8)))
    LAST_RESULT = res

    acc = np.zeros((R, H), dtype=np.float32)
    for c in range(8):
        blocks = res.results[c]["outp"]  # [n_chunks, P, NCH//P, H] bf16
        for ci, (_, r0, nch, _) in enumerate(chunks):
            rowsmaj = blocks[ci].transpose(1, 0, 2).reshape(NCH, H)[:nch]
            acc[r0 : r0 + nch] += rowsmaj.astype(np.float32)
    return acc
